# revision 14
# baseline (speedup 1.0000x reference)
"""Trainium2 Bass kernel for multi-head cross-attention.

Problem: q [4, 2048, 512], kv [4, 4096, 128], 8 heads x 64 dim,
out = softmax((q Wq^T)(kv Wk^T)^T / 8) (kv Wv^T) Wo^T + b_o.

Sharding: pure data parallel over 8 NeuronCores; core c handles batch c//2,
query rows (c%2)*1024:(c%2+1)*1024. No collectives.

Per-core dataflow (all layouts feature-major so the PE contracts on partitions):
  - PE-transpose q, kv and the weights into SBUF (bf16).
  - Projections QT=[e,i], KT=[e,j], V=[j,e] via bf16 matmuls.
  - Per head: S^T[j,i] = KT_h^T-free matmul; ACT exp (scale=1/8 folded in,
    no max-subtract needed: logits ~ N(0,1), fp32 exp is safe);
    AV uses V augmented with a ones column so the softmax denominator
    falls out of the same matmul (row 64 of the [65, i] accumulator).
  - Division by the denominator happens after AV (it commutes), via DVE
    reciprocal + SBUF->SBUF DMA partition-broadcast + tensor_mul.
  - out_linear accumulates 8 K=64 matmuls; bias added on DVE.
"""

import sys

import numpy as np

for _p in ("/opt/trn_rl_repo",):
    if _p not in sys.path:
        sys.path.insert(0, _p)

import concourse.bass as bass
import concourse.tile as tile
from concourse import mybir
from concourse.bass_utils import run_bass_kernel_spmd

F32 = mybir.dt.float32
BF16 = mybir.dt.bfloat16

B = 4
NQ_FULL = 2048
NQ = 1024          # queries per core
NK = 4096
C = 512            # q feature dim
KVC = 128          # kv feature dim
H = 8
D = 64
INNER = H * D      # 512
SCALE = D ** -0.5  # 0.125
N_CORES = 8
DEBUG_DUMP = False


def _emit(tc, nc, q, kv, w_q, w_k, w_v, w_o, b_o, out):
    from concourse.masks import make_identity

    Exp = mybir.ActivationFunctionType.Exp

    with (
        tc.tile_pool(name="const", bufs=1) as const,
        tc.tile_pool(name="stage", bufs=4) as stage,
        tc.tile_pool(name="big", bufs=1) as big,
        tc.tile_pool(name="epool", bufs=4) as epool,
        tc.tile_pool(name="hdr", bufs=2) as hdr,
        tc.tile_pool(name="outp", bufs=2) as outp,
        tc.tile_pool(name="dscr", bufs=2, space="DRAM") as dscr,
        tc.tile_pool(name="ps", bufs=2, space="PSUM") as ps,
        tc.tile_pool(name="avps", bufs=2, space="PSUM") as avps,
    ):
        identity = const.tile([128, 128], F32)
        make_identity(nc, identity)

        # Layouts (feature-major so the PE contracts over partitions):
        wqT = big.tile([128, 4, C], BF16)      # [c_in, cb, e]
        wkT = big.tile([128, C], BF16)         # [c, e]
        wvT = big.tile([128, C], BF16)         # [c, e]
        qTin = big.tile([128, 4, NQ], BF16)    # [c_in, cb, i]
        kvT = big.tile([128, NK], BF16)        # [c, j]
        KT = big.tile([128, 4, NK], BF16)      # [e_in, et, j]
        QT = big.tile([128, 4, NQ], BF16)      # [e_in, et, i]
        vaug = big.tile([128, 32, H, 65], BF16)  # [j_in, jc, h, d|ones]
        woT128 = big.tile([128, 4, C], BF16)   # [e_in, hp, o]
        aoT128 = big.tile([128, 4, NQ], BF16)  # attn-out^T packed head pairs

        def transpose_block(src, dst):
            # src: SBUF [128, m] f32 -> PE transpose -> dst: SBUF [m, 128] bf16
            m = src.shape[-1]
            trp = ps.tile([128, 128], F32, tag="sx", name="trp")
            nc.tensor.transpose(trp[:m, :], src, identity)
            nc.vector.tensor_copy(dst, trp[:m, :])

        # ---- kv + w_k: f32 loads on the two HWDGE rings, PE transposes ----
        for ch in range(4):
            kv_t = stage.tile([128, 8, KVC], F32, tag="kvst", bufs=2,
                              name="kv_t")
            nc.sync.dma_start(
                out=kv_t,
                in_=kv[ch * 1024:(ch + 1) * 1024, :].rearrange(
                    "(a p) d -> p a d", p=128))
            for a in range(8):
                jb = ch * 8 + a
                transpose_block(kv_t[:, a, :], kvT[:, jb * 128:(jb + 1) * 128])
        wk_t = stage.tile([128, 4, KVC], F32, tag="wkv", name="wk_t")
        nc.scalar.dma_start(out=wk_t,
                            in_=w_k.rearrange("(eb p) c -> p eb c", p=128))
        for eb in range(4):
            transpose_block(wk_t[:, eb, :], wkT[:, eb * 128:(eb + 1) * 128])

        # ---- KT projection ----
        for et in range(4):
            for jc in range(8):
                kt_ps = ps.tile([128, 512], F32, tag="sx", name="kt_ps")
                nc.tensor.matmul(kt_ps,
                                 lhsT=wkT[:, et * 128:(et + 1) * 128],
                                 rhs=kvT[:, jc * 512:(jc + 1) * 512],
                                 start=True, stop=True)
                nc.vector.tensor_copy(KT[:, et, jc * 512:(jc + 1) * 512], kt_ps)

        # ---- q + w_q loads + transposes, QT projection ----
        for ib in range(8):
            q_t = stage.tile([128, C], F32, tag="stage", name="q_t")
            eng = nc.scalar if ib % 2 else nc.sync
            eng.dma_start(out=q_t, in_=q[ib * 128:(ib + 1) * 128, :])
            for cb in range(4):
                transpose_block(q_t[:, cb * 128:(cb + 1) * 128],
                                qTin[:, cb, ib * 128:(ib + 1) * 128])
        for eb in range(4):
            wq_t = stage.tile([128, C], F32, tag="stage", name="wq_t")
            eng = nc.scalar if eb % 2 else nc.sync
            eng.dma_start(out=wq_t, in_=w_q[eb * 128:(eb + 1) * 128, :])
            for cb in range(4):
                transpose_block(wq_t[:, cb * 128:(cb + 1) * 128],
                                wqT[:, cb, eb * 128:(eb + 1) * 128])
        for et in range(4):
            for ih in range(2):
                qt_ps = ps.tile([128, 512], F32, tag="sx", name="qt_ps")
                for cb in range(4):
                    nc.tensor.matmul(qt_ps,
                                     lhsT=wqT[:, cb, et * 128:(et + 1) * 128],
                                     rhs=qTin[:, cb, ih * 512:(ih + 1) * 512],
                                     start=(cb == 0), stop=(cb == 3))
                nc.vector.tensor_copy(QT[:, et, ih * 512:(ih + 1) * 512], qt_ps)

        # ---- w_v + V projection ----
        wv_t = stage.tile([128, 4, KVC], F32, tag="wkv", name="wv_t")
        nc.scalar.dma_start(out=wv_t,
                            in_=w_v.rearrange("(eb p) c -> p eb c", p=128))
        for eb in range(4):
            transpose_block(wv_t[:, eb, :], wvT[:, eb * 128:(eb + 1) * 128])
        nc.vector.memset(vaug[:, :, :, 64:65], 1.0)
        for jc in range(32):
            v_ps = ps.tile([128, 512], F32, tag="sx", name="v_ps")
            nc.tensor.matmul(v_ps,
                             lhsT=kvT[:, jc * 128:(jc + 1) * 128],
                             rhs=wvT,
                             start=True, stop=True)
            nc.vector.tensor_copy(vaug[:, jc, :, 0:64],
                                  v_ps.rearrange("p (h d) -> p h d", h=H))

        # ---- w_o + bias: loaded now, consumed by out_linear at the end ----
        for ob in range(4):
            wo_t = stage.tile([128, C], F32, tag="stage", name="wo_t")
            eng = nc.scalar if ob % 2 else nc.sync
            eng.dma_start(out=wo_t, in_=w_o[ob * 128:(ob + 1) * 128, :])
            for eb in range(4):
                transpose_block(wo_t[:, eb * 128:(eb + 1) * 128],
                                woT128[:, eb, ob * 128:(ob + 1) * 128])
        bias_bc = const.tile([128, C], F32)
        nc.gpsimd.dma_start(out=bias_bc, in_=b_o.to_broadcast([128, C]))

        # ---- attention: head pairs; S^T matmuls interleaved across the two
        # 64-row groups so they run concurrently on the PE ----
        def head_tail(hp, k, av_ps):
            h = 2 * hp + k
            # free the PSUM accumulator quickly with one [65, i] copy
            avsb = hdr.tile([65, NQ], F32, tag="avsb", name="avsb")
            nc.vector.tensor_copy(avsb, av_ps)
            rdram = dscr.tile([1, NQ], F32, name="rdram")
            nc.gpsimd.dma_start(out=rdram, in_=avsb[64:65, :])
            rbc = hdr.tile([64, NQ], F32, tag="rbc", name="rbc")
            nc.gpsimd.dma_start(out=rbc, in_=rdram.to_broadcast([64, NQ]))
            rbcr = hdr.tile([64, NQ], F32, tag="rbcr", name="rbcr")
            nc.vector.reciprocal_approx_fast(rbcr, rbc)
            if k == 0:
                nc.vector.tensor_mul(aoT128[0:64, hp, :], avsb[0:64, :], rbcr)
            else:
                aodd = hdr.tile([64, NQ], BF16, tag="aodd", name="aodd")
                nc.vector.tensor_mul(aodd, avsb[0:64, :], rbcr)
                ascr = dscr.tile([64, NQ], BF16, name="ascr")
                nc.gpsimd.dma_start(out=ascr, in_=aodd)
                nc.gpsimd.dma_start(out=aoT128[64:128, hp, :], in_=ascr)

        for hp in range(4):
            et = hp
            avs = [avps.tile([65, NQ], F32, name="av_ps") for _ in range(2)]
            for jc in range(32):
                sts = [ps.tile([128, NQ], F32, tag="sx", name="st_ps")
                       for _ in range(2)]
                for ih in range(2):
                    for k in range(2):
                        row = k * 64
                        nc.tensor.matmul(
                            sts[k][:, ih * 512:(ih + 1) * 512],
                            lhsT=KT[row:row + 64, et, jc * 128:(jc + 1) * 128],
                            rhs=QT[row:row + 64, et, ih * 512:(ih + 1) * 512],
                            start=True, stop=True)
                for k in range(2):
                    expS = epool.tile([128, NQ], BF16, name="expS")
                    nc.scalar.activation(out=expS, in_=sts[k], func=Exp,
                                         scale=SCALE)
                    for ih in range(2):
                        nc.tensor.matmul(
                            avs[k][:, ih * 512:(ih + 1) * 512],
                            lhsT=vaug[:, jc, 2 * hp + k, :],
                            rhs=expS[:, ih * 512:(ih + 1) * 512],
                            start=(jc == 0), stop=(jc == 31))
            for k in range(2):
                head_tail(hp, k, avs[k])

        if DEBUG_DUMP:
            for nm, t in [("kvT", kvT), ("wkT", wkT), ("wvT", wvT),
                          ("qTin", qTin), ("wqT", wqT), ("KT", KT),
                          ("QT", QT), ("vaug", vaug), ("aoT128", aoT128),
                          ("woT128", woT128)]:
                dd = nc.dram_tensor("d_" + nm, list(t.shape), t.dtype,
                                    kind="ExternalOutput").ap()
                nc.gpsimd.dma_start(out=dd, in_=t)

        # ---- out linear: K=128 over packed head pairs ----
        for ic in range(8):
            ol_ps = ps.tile([128, 512], F32, tag="sx", name="ol_ps")
            for hp in range(4):
                nc.tensor.matmul(ol_ps,
                                 lhsT=aoT128[:, hp, ic * 128:(ic + 1) * 128],
                                 rhs=woT128[:, hp, :],
                                 start=(hp == 0), stop=(hp == 3))
            o_sb = outp.tile([128, C], F32, name="o_sb")
            nc.vector.tensor_add(o_sb, ol_ps, bias_bc)
            nc.gpsimd.dma_start(out=out[ic * 128:(ic + 1) * 128, :], in_=o_sb)


def build_program():
    from concourse import bacc
    nc = bacc.Bacc("TRN2", target_bir_lowering=False, debug=False)
    q = nc.dram_tensor("q", [NQ, C], F32, kind="ExternalInput").ap()
    kv = nc.dram_tensor("kv", [NK, KVC], F32, kind="ExternalInput").ap()
    w_q = nc.dram_tensor("w_q", [INNER, C], F32, kind="ExternalInput").ap()
    w_k = nc.dram_tensor("w_k", [INNER, KVC], F32, kind="ExternalInput").ap()
    w_v = nc.dram_tensor("w_v", [INNER, KVC], F32, kind="ExternalInput").ap()
    w_o = nc.dram_tensor("w_o", [C, INNER], F32, kind="ExternalInput").ap()
    b_o = nc.dram_tensor("b_o", [1, C], F32, kind="ExternalInput").ap()
    out = nc.dram_tensor("out", [NQ, C], F32, kind="ExternalOutput").ap()
    with tile.TileContext(nc) as tc:
        _emit(tc, nc, q, kv, w_q, w_k, w_v, w_o, b_o, out)
    nc.compile()
    return nc


def make_in_maps(q, kv, w_q, w_k, w_v, w_o, b_o):
    q = np.ascontiguousarray(q, dtype=np.float32)
    kv = np.ascontiguousarray(kv, dtype=np.float32)
    w_q = np.ascontiguousarray(w_q, dtype=np.float32)
    w_k = np.ascontiguousarray(w_k, dtype=np.float32)
    w_v = np.ascontiguousarray(w_v, dtype=np.float32)
    w_o = np.ascontiguousarray(w_o, dtype=np.float32)
    b_o = np.ascontiguousarray(b_o, dtype=np.float32).reshape(1, C)
    in_maps = []
    for core in range(N_CORES):
        b, half = core // 2, core % 2
        in_maps.append({
            "q": np.ascontiguousarray(q[b, half * NQ:(half + 1) * NQ]),
            "kv": kv[b],
            "w_q": w_q, "w_k": w_k, "w_v": w_v, "w_o": w_o, "b_o": b_o,
        })
    return in_maps


def assemble(results):
    out = np.zeros((B, NQ_FULL, C), np.float32)
    for core in range(N_CORES):
        b, half = core // 2, core % 2
        out[b, half * NQ:(half + 1) * NQ] = results[core]["out"]
    return out


def run(inputs, trace=False, **kwargs):
    nc = build_program()
    in_maps = make_in_maps(**inputs)
    res = run_bass_kernel_spmd(nc, in_maps, core_ids=list(range(N_CORES)),
                               trace=trace, **kwargs)
    return assemble(res.results), res


def kernel(q, kv, w_q, w_k, w_v, w_o, b_o):
    out, _ = run(dict(q=q, kv=kv, w_q=w_q, w_k=w_k, w_v=w_v, w_o=w_o, b_o=b_o))
    return out


# revision 15
# speedup vs baseline: 1.4785x; 1.4785x over previous
"""Trainium2 Bass kernel for multi-head cross-attention.

Problem: q [4, 2048, 512], kv [4, 4096, 128], 8 heads x 64 dim,
out = softmax((q Wq^T)(kv Wk^T)^T / 8) (kv Wv^T) Wo^T + b_o.

Sharding: pure data parallel over 8 NeuronCores; core c handles batch c//2,
query rows (c%2)*1024:(c%2+1)*1024. No collectives.

Per-core dataflow (all layouts feature-major so the PE contracts on partitions):
  - PE-transpose q, kv and the weights into SBUF (bf16).
  - Projections QT=[e,i], KT=[e,j], V=[j,e] via bf16 matmuls.
  - Per head: S^T[j,i] = KT_h^T-free matmul; ACT exp (scale=1/8 folded in,
    no max-subtract needed: logits ~ N(0,1), fp32 exp is safe);
    AV uses V augmented with a ones column so the softmax denominator
    falls out of the same matmul (row 64 of the [65, i] accumulator).
  - Division by the denominator happens after AV (it commutes), via DVE
    reciprocal + SBUF->SBUF DMA partition-broadcast + tensor_mul.
  - out_linear accumulates 8 K=64 matmuls; bias added on DVE.
"""

import sys

import numpy as np

for _p in ("/opt/trn_rl_repo",):
    if _p not in sys.path:
        sys.path.insert(0, _p)

import concourse.bass as bass
import concourse.tile as tile
from concourse import mybir
from concourse.bass_utils import run_bass_kernel_spmd

F32 = mybir.dt.float32
BF16 = mybir.dt.bfloat16

B = 4
NQ_FULL = 2048
NQ = 1024          # queries per core
NK = 4096
C = 512            # q feature dim
KVC = 128          # kv feature dim
H = 8
D = 64
INNER = H * D      # 512
SCALE = D ** -0.5  # 0.125
N_CORES = 8
DEBUG_DUMP = False


def _emit(tc, nc, q, kv, w_q, w_k, w_v, w_o, b_o, out):
    from concourse.masks import make_identity

    Exp = mybir.ActivationFunctionType.Exp

    with (
        tc.tile_pool(name="const", bufs=1) as const,
        tc.tile_pool(name="stage", bufs=4) as stage,
        tc.tile_pool(name="big", bufs=1) as big,
        tc.tile_pool(name="epool", bufs=4) as epool,
        tc.tile_pool(name="hdr", bufs=2) as hdr,
        tc.tile_pool(name="outp", bufs=2) as outp,
        tc.tile_pool(name="dscr", bufs=2, space="DRAM") as dscr,
        tc.tile_pool(name="ps", bufs=2, space="PSUM") as ps,
        tc.tile_pool(name="avps", bufs=2, space="PSUM") as avps,
    ):
        identity = const.tile([128, 128], F32)
        make_identity(nc, identity)

        # Layouts (feature-major so the PE contracts over partitions):
        wqT = big.tile([128, 4, C], BF16)      # [c_in, cb, e]
        wkT = big.tile([128, C], BF16)         # [c, e]
        wvT = big.tile([128, C], BF16)         # [c, e]
        qTin = big.tile([128, 4, NQ], BF16)    # [c_in, cb, i]
        kvT = big.tile([128, NK], BF16)        # [c, j]
        KT = big.tile([128, 4, NK], BF16)      # [e_in, et, j]
        QT = big.tile([128, 4, NQ], BF16)      # [e_in, et, i]
        vaug = big.tile([128, 32, H, 65], BF16)  # [j_in, jc, h, d|ones]
        woT128 = big.tile([128, 4, C], BF16)   # [e_in, hp, o]
        aoT128 = big.tile([128, 4, NQ], BF16)  # attn-out^T packed head pairs

        def transpose_group(srcs, dst):
            # srcs: list of SBUF [128, 128] f32 blocks -> one PSUM group ->
            # single strided DVE copy into dst ([128, len, 128] bf16 view)
            n = len(srcs)
            trp4 = ps.tile([128, 4, 128], F32, tag="sx", name="trp4")
            for t, s in enumerate(srcs):
                nc.tensor.transpose(trp4[:, t, :], s, identity)
            nc.vector.tensor_copy(dst, trp4[:, :n, :])

        # ---- kv + w_k: f32 loads on the two HWDGE rings, PE transposes ----
        for ch in range(4):
            kv_t = stage.tile([128, 8, KVC], F32, tag="kvst", bufs=2,
                              name="kv_t")
            nc.sync.dma_start(
                out=kv_t,
                in_=kv[ch * 1024:(ch + 1) * 1024, :].rearrange(
                    "(a p) d -> p a d", p=128))
            for g in range(2):
                jb = ch * 8 + g * 4
                transpose_group([kv_t[:, g * 4 + a4, :] for a4 in range(4)],
                                kvT[:, jb * 128:(jb + 4) * 128].rearrange(
                                    "p (a j) -> p a j", a=4))
        wk_t = stage.tile([128, 4, KVC], F32, tag="wkv", name="wk_t")
        nc.scalar.dma_start(out=wk_t,
                            in_=w_k.rearrange("(eb p) c -> p eb c", p=128))
        transpose_group([wk_t[:, eb, :] for eb in range(4)],
                        wkT.rearrange("p (eb e) -> p eb e", eb=4))

        # ---- KT projection ----
        for et in range(4):
            for jc in range(8):
                kt_ps = ps.tile([128, 512], F32, tag="sx", name="kt_ps")
                nc.tensor.matmul(kt_ps,
                                 lhsT=wkT[:, et * 128:(et + 1) * 128],
                                 rhs=kvT[:, jc * 512:(jc + 1) * 512],
                                 start=True, stop=True)
                nc.vector.tensor_copy(KT[:, et, jc * 512:(jc + 1) * 512], kt_ps)

        # ---- q + w_q loads + transposes, QT projection ----
        for ib in range(8):
            q_t = stage.tile([128, C], F32, tag="stage", name="q_t")
            eng = nc.scalar if ib % 2 else nc.sync
            eng.dma_start(out=q_t, in_=q[ib * 128:(ib + 1) * 128, :])
            transpose_group([q_t[:, cb * 128:(cb + 1) * 128]
                             for cb in range(4)],
                            qTin[:, :, ib * 128:(ib + 1) * 128])
        for eb in range(4):
            wq_t = stage.tile([128, C], F32, tag="stage", name="wq_t")
            eng = nc.scalar if eb % 2 else nc.sync
            eng.dma_start(out=wq_t, in_=w_q[eb * 128:(eb + 1) * 128, :])
            transpose_group([wq_t[:, cb * 128:(cb + 1) * 128]
                             for cb in range(4)],
                            wqT[:, :, eb * 128:(eb + 1) * 128])
        for et in range(4):
            for ih in range(2):
                qt_ps = ps.tile([128, 512], F32, tag="sx", name="qt_ps")
                for cb in range(4):
                    nc.tensor.matmul(qt_ps,
                                     lhsT=wqT[:, cb, et * 128:(et + 1) * 128],
                                     rhs=qTin[:, cb, ih * 512:(ih + 1) * 512],
                                     start=(cb == 0), stop=(cb == 3))
                nc.vector.tensor_copy(QT[:, et, ih * 512:(ih + 1) * 512], qt_ps)

        # ---- w_v + V projection ----
        wv_t = stage.tile([128, 4, KVC], F32, tag="wkv", name="wv_t")
        nc.scalar.dma_start(out=wv_t,
                            in_=w_v.rearrange("(eb p) c -> p eb c", p=128))
        transpose_group([wv_t[:, eb, :] for eb in range(4)],
                        wvT.rearrange("p (eb e) -> p eb e", eb=4))
        nc.vector.memset(vaug[:, :, :, 64:65], 1.0)
        for jc in range(32):
            v_ps = ps.tile([128, 512], F32, tag="sx", name="v_ps")
            nc.tensor.matmul(v_ps,
                             lhsT=kvT[:, jc * 128:(jc + 1) * 128],
                             rhs=wvT,
                             start=True, stop=True)
            nc.vector.tensor_copy(vaug[:, jc, :, 0:64],
                                  v_ps.rearrange("p (h d) -> p h d", h=H))

        # ---- w_o + bias: loaded now, consumed by out_linear at the end ----
        for ob in range(4):
            wo_t = stage.tile([128, C], F32, tag="stage", name="wo_t")
            eng = nc.scalar if ob % 2 else nc.sync
            eng.dma_start(out=wo_t, in_=w_o[ob * 128:(ob + 1) * 128, :])
            transpose_group([wo_t[:, eb * 128:(eb + 1) * 128]
                             for eb in range(4)],
                            woT128[:, :, ob * 128:(ob + 1) * 128])
        bias_bc = const.tile([128, C], F32)
        nc.gpsimd.dma_start(out=bias_bc, in_=b_o.to_broadcast([128, C]))

        # ---- attention: head pairs; S^T matmuls interleaved across the two
        # 64-row groups so they run concurrently on the PE ----
        def head_tail(hp, k, av_ps):
            h = 2 * hp + k
            # free the PSUM accumulator quickly with one [65, i] copy
            avsb = hdr.tile([65, NQ], F32, tag="avsb", name="avsb")
            nc.vector.tensor_copy(avsb, av_ps)
            rdram = dscr.tile([1, NQ], F32, name="rdram")
            nc.gpsimd.dma_start(out=rdram, in_=avsb[64:65, :])
            rbc = hdr.tile([64, NQ], F32, tag="rbc", name="rbc")
            nc.gpsimd.dma_start(out=rbc, in_=rdram.to_broadcast([64, NQ]))
            rbcr = hdr.tile([64, NQ], F32, tag="rbcr", name="rbcr")
            nc.vector.reciprocal_approx_fast(rbcr, rbc)
            if k == 0:
                nc.vector.tensor_mul(aoT128[0:64, hp, :], avsb[0:64, :], rbcr)
            else:
                aodd = hdr.tile([64, NQ], BF16, tag="aodd", name="aodd")
                nc.vector.tensor_mul(aodd, avsb[0:64, :], rbcr)
                ascr = dscr.tile([64, NQ], BF16, name="ascr")
                nc.gpsimd.dma_start(out=ascr, in_=aodd)
                nc.gpsimd.dma_start(out=aoT128[64:128, hp, :], in_=ascr)

        for hp in range(4):
            et = hp
            avs = [avps.tile([65, NQ], F32, name="av_ps") for _ in range(2)]
            for jc in range(32):
                exps = []
                for k in range(2):
                    row = k * 64
                    st_ps = ps.tile([128, NQ], F32, tag="sx", name="st_ps")
                    for ih in range(2):
                        nc.tensor.matmul(
                            st_ps[:, ih * 512:(ih + 1) * 512],
                            lhsT=KT[row:row + 64, et, jc * 128:(jc + 1) * 128],
                            rhs=QT[row:row + 64, et, ih * 512:(ih + 1) * 512],
                            start=True, stop=True)
                    expS = epool.tile([128, NQ], BF16, name="expS")
                    nc.scalar.activation(out=expS, in_=st_ps, func=Exp,
                                         scale=SCALE)
                    exps.append(expS)
                for k in range(2):
                    for ih in range(2):
                        nc.tensor.matmul(
                            avs[k][:, ih * 512:(ih + 1) * 512],
                            lhsT=vaug[:, jc, 2 * hp + k, :],
                            rhs=exps[k][:, ih * 512:(ih + 1) * 512],
                            start=(jc == 0), stop=(jc == 31))
            for k in range(2):
                head_tail(hp, k, avs[k])

        if DEBUG_DUMP:
            for nm, t in [("kvT", kvT), ("wkT", wkT), ("wvT", wvT),
                          ("qTin", qTin), ("wqT", wqT), ("KT", KT),
                          ("QT", QT), ("vaug", vaug), ("aoT128", aoT128),
                          ("woT128", woT128)]:
                dd = nc.dram_tensor("d_" + nm, list(t.shape), t.dtype,
                                    kind="ExternalOutput").ap()
                nc.gpsimd.dma_start(out=dd, in_=t)

        # ---- out linear: K=128 over packed head pairs ----
        for ic in range(8):
            ol_ps = ps.tile([128, 512], F32, tag="sx", name="ol_ps")
            for hp in range(4):
                nc.tensor.matmul(ol_ps,
                                 lhsT=aoT128[:, hp, ic * 128:(ic + 1) * 128],
                                 rhs=woT128[:, hp, :],
                                 start=(hp == 0), stop=(hp == 3))
            o_sb = outp.tile([128, C], F32, name="o_sb")
            nc.vector.tensor_add(o_sb, ol_ps, bias_bc)
            nc.gpsimd.dma_start(out=out[ic * 128:(ic + 1) * 128, :], in_=o_sb)


def build_program():
    from concourse import bacc
    nc = bacc.Bacc("TRN2", target_bir_lowering=False, debug=False)
    q = nc.dram_tensor("q", [NQ, C], F32, kind="ExternalInput").ap()
    kv = nc.dram_tensor("kv", [NK, KVC], F32, kind="ExternalInput").ap()
    w_q = nc.dram_tensor("w_q", [INNER, C], F32, kind="ExternalInput").ap()
    w_k = nc.dram_tensor("w_k", [INNER, KVC], F32, kind="ExternalInput").ap()
    w_v = nc.dram_tensor("w_v", [INNER, KVC], F32, kind="ExternalInput").ap()
    w_o = nc.dram_tensor("w_o", [C, INNER], F32, kind="ExternalInput").ap()
    b_o = nc.dram_tensor("b_o", [1, C], F32, kind="ExternalInput").ap()
    out = nc.dram_tensor("out", [NQ, C], F32, kind="ExternalOutput").ap()
    with tile.TileContext(nc) as tc:
        _emit(tc, nc, q, kv, w_q, w_k, w_v, w_o, b_o, out)
    nc.compile()
    return nc


def make_in_maps(q, kv, w_q, w_k, w_v, w_o, b_o):
    q = np.ascontiguousarray(q, dtype=np.float32)
    kv = np.ascontiguousarray(kv, dtype=np.float32)
    w_q = np.ascontiguousarray(w_q, dtype=np.float32)
    w_k = np.ascontiguousarray(w_k, dtype=np.float32)
    w_v = np.ascontiguousarray(w_v, dtype=np.float32)
    w_o = np.ascontiguousarray(w_o, dtype=np.float32)
    b_o = np.ascontiguousarray(b_o, dtype=np.float32).reshape(1, C)
    in_maps = []
    for core in range(N_CORES):
        b, half = core // 2, core % 2
        in_maps.append({
            "q": np.ascontiguousarray(q[b, half * NQ:(half + 1) * NQ]),
            "kv": kv[b],
            "w_q": w_q, "w_k": w_k, "w_v": w_v, "w_o": w_o, "b_o": b_o,
        })
    return in_maps


def assemble(results):
    out = np.zeros((B, NQ_FULL, C), np.float32)
    for core in range(N_CORES):
        b, half = core // 2, core % 2
        out[b, half * NQ:(half + 1) * NQ] = results[core]["out"]
    return out


def run(inputs, trace=False, **kwargs):
    nc = build_program()
    in_maps = make_in_maps(**inputs)
    res = run_bass_kernel_spmd(nc, in_maps, core_ids=list(range(N_CORES)),
                               trace=trace, **kwargs)
    return assemble(res.results), res


def kernel(q, kv, w_q, w_k, w_v, w_o, b_o):
    out, _ = run(dict(q=q, kv=kv, w_q=w_q, w_k=w_k, w_v=w_v, w_o=w_o, b_o=b_o))
    return out
